# revision 7
# baseline (speedup 1.0000x reference)
"""Trainium2 Bass kernel for nn_Encoder (SimpleRNN encoder).

reference:
    x_emb = emb[x]                                  # [B, T, E]
    xin   = einsum('bte,eu->tbu', x_emb, Wx) + b    # [T, B, U]
    h_t   = tanh(xin_t + h_{t-1} @ Wh)              # scan over T
    returns (outputs transposed to [B, T, U], final state [B, U])

Sharding: data-parallel over batch across 8 cores (8 batches per core).
Wx/Wh/b/emb replicated.

Per-core dataflow:
  Phase 1: indirect-DMA gather of emb rows (128 tokens/tile), PE transpose
  to put E on partitions, matmul with Wx + bias -> xin staged in DRAM.
  Phase 2 (recurrence): h kept in layout H[32j+b, c] = h[b, 256j+c].
  Each step: 4 col-group matmul chains (tile_position=(0,32j), M=32,
  N=256, 8 K-tiles of 128) + a small identity-matmul that adds xin_t into
  the same PSUM accumulation; tanh on ACT; DVE StreamTranspose (32x32
  blocks) rebuilds the transposed h (lhsT tiles) for the next step.
"""

import os
from contextlib import ExitStack

import numpy as np

import concourse.bass as bass
import concourse.tile as tile
from concourse import bacc, mybir
from concourse.bass_utils import run_bass_kernel_spmd

VOCAB, EMB, UNITS, B, T = 32000, 256, 1024, 64, 512
NCORES = 8
BL = B // NCORES            # batches per core
TOK_TILES = BL * T // 128   # 32 token tiles of 128 tokens per core
TI = 128 // BL              # timesteps per token tile = 16

f32 = mybir.dt.float32
i32 = mybir.dt.int32


def _build(dt_mm_name: str, t_steps: int):
    """Build the single-core SPMD program. Returns the compiled Bass object."""
    dmm = mybir.dt.float32r if dt_mm_name == "f32r" else mybir.dt.float32
    Tanh = mybir.ActivationFunctionType.Tanh

    nc = bacc.Bacc("TRN2", target_bir_lowering=False, debug=False,
                   num_devices=NCORES)

    x_l = nc.dram_tensor("x_l", [BL, T], i32, kind="ExternalInput")
    hidden_l = nc.dram_tensor("hidden_l", [BL, UNITS], f32, kind="ExternalInput")
    emb = nc.dram_tensor("emb", [VOCAB, EMB], f32, kind="ExternalInput")
    Wx = nc.dram_tensor("Wx", [EMB, UNITS], f32, kind="ExternalInput")
    Wh = nc.dram_tensor("Wh", [UNITS, UNITS], f32, kind="ExternalInput")
    bias = nc.dram_tensor("bias", [1, UNITS], f32, kind="ExternalInput")
    E_in = nc.dram_tensor("E_in", [BL, 128], f32, kind="ExternalInput")
    I_in = nc.dram_tensor("I_in", [128, 128], f32, kind="ExternalInput")
    out_l = nc.dram_tensor("out_l", [BL, T, UNITS], f32, kind="ExternalOutput")
    state_l = nc.dram_tensor("state_l", [BL, UNITS], f32, kind="ExternalOutput")

    n_tok_tiles = BL * t_steps // 128 if t_steps >= TI else 1

    with tile.TileContext(nc) as tc, ExitStack() as ctx:
        const = ctx.enter_context(tc.tile_pool(name="const", bufs=1))
        ph1 = ctx.enter_context(tc.tile_pool(name="ph1", bufs=3))
        work = ctx.enter_context(tc.tile_pool(name="work", bufs=3))
        xinp = ctx.enter_context(tc.tile_pool(name="xinp", bufs=4))
        psT = ctx.enter_context(tc.tile_pool(name="psT", bufs=2, space="PSUM"))
        psX = ctx.enter_context(tc.tile_pool(name="psX", bufs=4, space="PSUM"))
        psR = ctx.enter_context(tc.tile_pool(name="psR", bufs=2, space="PSUM"))
        xdram = ctx.enter_context(tc.tile_pool(name="xdram", bufs=n_tok_tiles,
                                               space="DRAM"))

        # ---- constants into SBUF ----
        Wh_sb = const.tile([128, 8 * UNITS], f32)
        for p in range(4):
            for k in range(8):
                nc.sync.dma_start(
                    Wh_sb[32 * p:32 * p + 32, UNITS * k:UNITS * (k + 1)],
                    Wh[256 * p + 32 * k:256 * p + 32 * k + 32, :])
        Wx_sb = const.tile([128, 2 * UNITS], f32)
        for q in range(2):
            nc.sync.dma_start(Wx_sb[:, UNITS * q:UNITS * (q + 1)],
                              Wx[128 * q:128 * q + 128, :])
        E_sb = const.tile([BL, 128], f32)
        nc.sync.dma_start(E_sb[:], E_in[:])
        I_sb = const.tile([128, 128], f32)
        nc.sync.dma_start(I_sb[:], I_in[:])
        b_sb = const.tile([1, UNITS], f32)
        nc.sync.dma_start(b_sb[:], bias[:])
        ones_sb = const.tile([1, 128], f32)
        nc.gpsimd.memset(ones_sb[:], 1.0)
        idx_all = const.tile([128, TOK_TILES], i32)
        x_tm = x_l.rearrange("b (m ti) -> b ti m", ti=TI)
        for bb in range(BL):
            nc.sync.dma_start(idx_all[TI * bb:TI * (bb + 1), :], x_tm[bb])

        # ---- phase 1: gather + xin = x_emb @ Wx + b ----
        xin_tiles = []
        for m in range(n_tok_tiles):
            xd = xdram.tile([128, UNITS], f32, tag=f"xin{m}")
            xin_tiles.append(xd)
            xe = ph1.tile([128, EMB], f32, tag="xe")
            nc.gpsimd.indirect_dma_start(
                out=xe[:], out_offset=None, in_=emb[:],
                in_offset=bass.IndirectOffsetOnAxis(ap=idx_all[:, m:m + 1],
                                                    axis=0))
            xeT = ph1.tile([128, EMB], f32, tag="xeT")
            for q in range(2):
                pT = psT.tile([128, 128], f32, tag="pT")
                nc.tensor.transpose(pT[:], xe[:, 128 * q:128 * q + 128],
                                    I_sb[:])
                nc.vector.tensor_copy(xeT[:, 128 * q:128 * q + 128], pT[:])
            for h2 in range(2):
                px = psX.tile([128, 512], f32, tag="px")
                for q in range(2):
                    nc.tensor.matmul(
                        px[:],
                        xeT[:, 128 * q:128 * q + 128].bitcast(dmm),
                        Wx_sb[:, UNITS * q + 512 * h2:
                              UNITS * q + 512 * h2 + 512].bitcast(dmm),
                        start=(q == 0), stop=False)
                nc.tensor.matmul(
                    px[:], ones_sb[:, :128].bitcast(dmm),
                    b_sb[:, 512 * h2:512 * h2 + 512].bitcast(dmm),
                    start=False, stop=True)
                sx = ph1.tile([128, 512], f32, tag="sx")
                nc.scalar.copy(sx[:], px[:])
                nc.sync.dma_start(xd[:, 512 * h2:512 * h2 + 512], sx[:])

        # ---- phase 2: recurrence ----
        # H[32j + b, c] = h[b, 256j + c]; hT[32p + a, 32k + b] = h[b, 256p+32k+a]
        H = work.tile([128, 256], f32, tag="H")
        nc.gpsimd.memset(H[:], 0.0)
        for j in range(4):
            nc.sync.dma_start(H[32 * j:32 * j + BL, :],
                              hidden_l[:, 256 * j:256 * j + 256])
        hT = work.tile([128, 256], f32, tag="hT")
        nc.vector.transpose(hT[:], H[:])

        for t in range(t_steps):
            m, ti = divmod(t, TI)
            xin_t = xinp.tile([BL, UNITS], f32, tag="xin_t")
            nc.sync.dma_start(
                xin_t[:],
                xin_tiles[m].rearrange("(b ti) u -> b ti u", ti=TI)[:, ti, :])

            ps_t = psR.tile([128, 256], f32, tag="ps_t")
            for j in range(4):
                for k in range(8):
                    nc.tensor.matmul(
                        ps_t[32 * j:32 * j + 32, :],
                        hT[:, 32 * k:32 * k + 32].bitcast(dmm),
                        Wh_sb[:, UNITS * k + 256 * j:
                              UNITS * k + 256 * j + 256].bitcast(dmm),
                        start=(k == 0), stop=False,
                        tile_position=(0, 32 * j))
                nc.tensor.matmul(
                    ps_t[32 * j:32 * j + 32, :],
                    E_sb[:, 32 * j:32 * j + 32].bitcast(dmm),
                    xin_t[:, 256 * j:256 * j + 256].bitcast(dmm),
                    start=False, stop=True, tile_position=(0, 32 * j))

            Hn = work.tile([128, 256], f32, tag="H")
            hTn = work.tile([128, 256], f32, tag="hT")
            for c2 in range(2):
                sl = slice(128 * c2, 128 * c2 + 128)
                nc.scalar.activation(Hn[:, sl], ps_t[:, sl], Tanh)
                nc.vector.transpose(hTn[:, sl], Hn[:, sl])
            for j in range(4):
                nc.sync.dma_start(out_l[:, t, 256 * j:256 * j + 256],
                                  Hn[32 * j:32 * j + BL, :])
            H, hT = Hn, hTn

        for j in range(4):
            nc.sync.dma_start(state_l[:, 256 * j:256 * j + 256],
                              H[32 * j:32 * j + BL, :])

    nc.compile()
    return nc


def _make_consts():
    E_np = np.zeros((BL, 128), np.float32)
    for j in range(4):
        for bb in range(BL):
            E_np[bb, 32 * j + bb] = 1.0
    I_np = np.eye(128, dtype=np.float32)
    return E_np, I_np


def kernel(x, hidden, emb, Wx, Wh, b):
    x = np.ascontiguousarray(np.asarray(x).astype(np.int32))
    hidden = np.ascontiguousarray(np.asarray(hidden, dtype=np.float32))
    emb = np.ascontiguousarray(np.asarray(emb, dtype=np.float32))
    Wx = np.ascontiguousarray(np.asarray(Wx, dtype=np.float32))
    Wh = np.ascontiguousarray(np.asarray(Wh, dtype=np.float32))
    b = np.ascontiguousarray(np.asarray(b, dtype=np.float32)).reshape(1, UNITS)

    dt_mm = os.environ.get("RNN_DT_MM", "f32r")
    t_steps = int(os.environ.get("RNN_T_STEPS", str(T)))

    nc = _build(dt_mm, t_steps)

    E_np, I_np = _make_consts()
    in_maps = []
    for c in range(NCORES):
        sl = slice(BL * c, BL * (c + 1))
        in_maps.append({
            "x_l": x[sl], "hidden_l": hidden[sl], "emb": emb,
            "Wx": Wx, "Wh": Wh, "bias": b, "E_in": E_np, "I_in": I_np,
        })

    res = run_bass_kernel_spmd(nc, in_maps, core_ids=list(range(NCORES)),
                               trace=False)

    output = np.empty((B, T, UNITS), np.float32)
    state = np.empty((B, UNITS), np.float32)
    for c in range(NCORES):
        sl = slice(BL * c, BL * (c + 1))
        output[sl] = res.results[c]["out_l"]
        state[sl] = res.results[c]["state_l"]
    return output, state
